# revision 54
# baseline (speedup 1.0000x reference)
"""Trainium2 Bass kernel for an 8-head attention layer (B=4, T=2048, K=512, H=8).

Sharding: DP=4 over batch x TP=2 over heads across 8 NeuronCores.
Core c handles batch c//2 with heads [4*(c%2), 4*(c%2)+4). Each core
produces a full [T, K] bf16 partial (its 4 heads' contribution + bias/2);
the host sums the pair partials during unshard -- no collectives at all.

Algebraic folding (host side):
  scores = (x Wq)(x Wk)^T / sqrt(K) = x M x^T  with M_h = Wq_h Wk_h^T / sqrt(K)
  out    = sum_h A_h x G_h              with G_h = Wv_h Wu_h
The unify stage is folded into G; the attention-value product is computed
as yT = (E-1)^T-contracted x8 on the tensor engine in fp8 DoubleRow (2x PE
throughput), then out_h = (yT + colsum(x8))^T G_h / Z. Centering the
exponentials at 1 (E-1 instead of E) cuts the fp8 quantization noise ~3x;
the exact correction is the rank-1 term colsum(x8) = sum_s x8[s,:], added
for free as a per-partition scalar when copying yT out of PSUM.

fp8 path: x8 = fp8(32 x) in two layouts (feature-major for scores,
key-major for A@x), q8 = fp8(1024 x M_h) cast out of PSUM on the scalar
engine, e2 = fp8(exp(s) - 1) centered on the gpsimd engine. Everything
else is bf16 with f32 PSUM accumulation.

Pipeline: per (qb, h) unit the PE does [32 scores DR MMs, interleaved with
the previous unit's Z/G/output work] then [32 A@x DR MMs kc-major]. The
scores window must out-last the scalar engine's 16 exp activations; the
interleaved G work provides that slack. PSUM: scores 3 + yT 2 + out2 2 +
Z 1 = 8 banks.
"""

import numpy as np
import ml_dtypes

# Problem constants (hardcoded; kernel.py must be self-contained).
B, T, K, H = 4, 2048, 512, 8
NCORES = 8
HL = H // 2        # heads per core (TP=2)
P = 128
KC = K // P        # feature chunks = 4
TC = T // P        # token/key chunks of 128 = 16
QB = 4             # query blocks of 512
NQC = 4            # 128-query subchunks per block
NPAIR = TC // 2    # key chunk pairs = 8
SX = 32.0          # x fp8 scale
SQ = 1024.0        # q' fp8 scale
ESC = 1.0 / (SX * SQ)

_NC_CACHE = {}


def _build_nc():
    import concourse.mybir as mybir
    import concourse.tile as tile
    from concourse import bacc

    f32 = mybir.dt.float32
    bf16 = mybir.dt.bfloat16
    fp8 = mybir.dt.float8e4
    Exp = mybir.ActivationFunctionType.Exp
    Copy = mybir.ActivationFunctionType.Copy
    DR = mybir.MatmulPerfMode.DoubleRow
    MUL = mybir.AluOpType.mult
    ADD = mybir.AluOpType.add

    nc = bacc.Bacc("TRN2", target_bir_lowering=False, debug=False,
                   num_devices=NCORES)

    # Contiguous per-partition layouts (host pre-arranged).
    xT_d = nc.dram_tensor("xTc", [P, QB, KC, 512], bf16, kind="ExternalInput")
    x8f_d = nc.dram_tensor("x8f", [P, KC, T], fp8, kind="ExternalInput")
    x8k_d = nc.dram_tensor("x8k", [P, TC, K], fp8, kind="ExternalInput")
    wm_d = nc.dram_tensor("wmc", [P, KC, HL * K], bf16, kind="ExternalInput")
    wg_d = nc.dram_tensor("wgc", [P, KC, HL * K], bf16, kind="ExternalInput")
    cs_d = nc.dram_tensor("csum", [P, KC], f32, kind="ExternalInput")
    bias_d = nc.dram_tensor("bias_bc", [P, K], f32, kind="ExternalInput")
    out_d = nc.dram_tensor("out", [T, K], bf16, kind="ExternalOutput")

    with tile.TileContext(nc) as tc:
        with (
            tc.tile_pool(name="const", bufs=1) as constp,
            tc.tile_pool(name="big", bufs=1) as bigp,
            tc.tile_pool(name="qkv", bufs=1) as qkvp,
            tc.tile_pool(name="attn", bufs=4) as attnp,
            tc.tile_pool(name="outp", bufs=2) as outp,
            tc.tile_pool(name="ps_mm", bufs=3, space="PSUM") as ps_mm,
            tc.tile_pool(name="ps_yt", bufs=2, space="PSUM") as ps_yt,
            tc.tile_pool(name="ps_o", bufs=2, space="PSUM") as ps_o,
            tc.tile_pool(name="ps_z", bufs=1, space="PSUM") as ps_z,
        ):
            # Warmup: keep the PE busy during the startup DMAs so the HAM
            # clock gate opens (1.2 -> 2.4 GHz) before real matmuls begin.
            dummy = constp.tile([P, 512], bf16)
            nc.vector.memset(dummy[:], 0.0)
            for w in range(24):
                wps = ps_mm.tile([P, 512], f32, tag="mm")
                nc.tensor.matmul(wps[:], dummy[:, :P], dummy[:],
                                 start=True, stop=True)

            ones = constp.tile([P, 1], bf16)
            nc.vector.memset(ones[:], 1.0)
            bias_sb = constp.tile([P, K], f32)
            nc.scalar.dma_start(bias_sb[:], bias_d[:, :])
            csum_sb = constp.tile([P, KC], f32)
            nc.scalar.dma_start(csum_sb[:], cs_d[:, :])

            # Startup loads: split every tensor across BOTH hardware DGE
            # queues by partition halves so they stream in parallel, ordered
            # by when the projection loop needs them. G rides the slow
            # software-DGE (gpsimd) queue (not needed until attention).
            HP = P // 2
            wm_sb = bigp.tile([P, KC, HL * K], bf16)
            xT_sbs = [bigp.tile([P, KC, 512], bf16, name=f"xT{tb}",
                                tag=f"xT{tb}") for tb in range(QB)]

            def split_dma(dst, src):
                nc.sync.dma_start(dst[:HP], src[:HP])
                nc.scalar.dma_start(dst[HP:], src[HP:])

            # HW queues carry only what the projection needs, in consumption
            # order; M for heads 1-3 and G ride the software (gpsimd) queue.
            split_dma(xT_sbs[0], xT_d.ap()[:, 0, :, :])
            split_dma(wm_sb[:, :, :P], wm_d.ap()[:, :, :P])
            split_dma(wm_sb[:, :, P:K], wm_d.ap()[:, :, P:K])
            for tb in range(1, QB):
                split_dma(xT_sbs[tb], xT_d.ap()[:, tb, :, :])
            split_dma(wm_sb[:, :, K:2 * K], wm_d.ap()[:, :, K:2 * K])
            split_dma(wm_sb[:, :, 2 * K:], wm_d.ap()[:, :, 2 * K:])
            wg_sb = bigp.tile([P, KC, HL * K], bf16)
            nc.gpsimd.dma_start(wg_sb[:], wg_d.ap()[:, :, :])
            x8f_sb = bigp.tile([P, KC, T], fp8)
            x8k_sb = bigp.tile([P, TC, K], fp8)

            # ---- projection phase: q8_h = fp8(1024 * x M_h) ----
            q8s = []
            for h in range(HL):
                q8_t = qkvp.tile([P, KC, T], fp8, name=f"q8_{h}", tag=f"q8_{h}")
                q8s.append(q8_t)
                for tb in range(QB):
                    if h == 0 and tb > 0:
                        # Filler matmuls: if a startup DMA is late the PE
                        # idles here; keep it busy so HAM stays unthrottled.
                        for w in range(4):
                            wps = ps_mm.tile([P, 512], f32, tag="mm")
                            nc.tensor.matmul(wps[:], dummy[:, :P], dummy[:],
                                             start=True, stop=True)
                    for dc in range(KC):
                        ps = ps_mm.tile([P, 512], f32, tag="mm")
                        col = h * K + dc * P
                        for kc in range(KC):
                            nc.tensor.matmul(
                                ps[:],
                                wm_sb[:, kc, col:col + P],
                                xT_sbs[tb][:, kc, :],
                                start=(kc == 0), stop=(kc == KC - 1))
                        nc.scalar.activation(
                            q8_t[:, dc, tb * 512:(tb + 1) * 512], ps[:],
                            Copy, scale=SQ)
                        if h == 0 and tb == 0 and dc == 0:
                            # Trigger the attention-phase loads once compute
                            # is rolling (keeps startup DMA rings free).
                            nc.scalar.dma_start(x8f_sb[:], x8f_d.ap()[:, :, :])
                            nc.sync.dma_start(x8k_sb[:], x8k_d.ap()[:, :, :])

            # ---- attention phase ----
            out_accs = {}

            def flush(st, part):
                # Previous unit's epilogue, interleaved into the current
                # unit's scores window: Z reduction + reciprocal + two
                # out2 = yT G_h chains (part 0), then the other two chains
                # (part 1), each followed by normalize-accumulate.
                h, qb, yt_sb, sum_acc, rv_box = st
                oacc = out_accs[qb]
                last = h == HL - 1
                if part == 0:
                    # Z per query: partition-dim reduction of sum_acc.
                    sps = ps_z.tile([P, NQC], f32, tag="z")
                    for qc in range(NQC):
                        nc.tensor.matmul(
                            sps[:, qc:qc + 1],
                            sum_acc[:, qc * P:(qc + 1) * P],
                            ones[:, :1], start=True, stop=True)
                    rv = attnp.tile([P, NQC], f32, tag="rinv", bufs=2)
                    nc.vector.reciprocal(rv[:], sps[:])
                    rv_box.append(rv)
                    qcs = (0, 1)
                else:
                    qcs = (2, 3)
                rv = rv_box[0]
                for qc in qcs:
                    ops = ps_o.tile([P, 512], f32, tag="o")
                    for kc in range(KC):
                        nc.tensor.matmul(
                            ops[:],
                            yt_sb[:, kc, qc * P:(qc + 1) * P],
                            wg_sb[:, kc, h * K:(h + 1) * K],
                            start=(kc == 0), stop=(kc == KC - 1))
                    in1 = bias_sb[:] if h == 0 else oacc[:, qc, :]
                    if last:
                        oacc8 = outp.tile([P, K], bf16, tag="oacc8", bufs=4)
                        nc.vector.scalar_tensor_tensor(
                            oacc8[:], ops[:], rv[:, qc:qc + 1], in1, MUL, ADD)
                        row = qb * 512 + qc * P
                        nc.sync.dma_start(out_d.ap()[row:row + P, :],
                                          oacc8[:])
                    else:
                        nc.vector.scalar_tensor_tensor(
                            oacc[:, qc, :], ops[:], rv[:, qc:qc + 1], in1,
                            MUL, ADD)

            prev = None
            for qb in range(QB):
                qsl = slice(qb * 512, (qb + 1) * 512)
                out_accs[qb] = outp.tile([P, NQC, K], f32, tag="oacc",
                                         name=f"oacc{qb}")
                for h in range(HL):
                    e2s = []
                    sum_acc = attnp.tile([P, 512], bf16, tag="sacc", bufs=2)
                    # scores for 8 key-chunk pairs, with the previous
                    # unit's epilogue interleaved at pairs 2 and 5.
                    for m in range(NPAIR):
                        e2_t = attnp.tile([P, 2, 512], fp8, tag="e2",
                                          bufs=10)
                        e2s.append(e2_t)
                        for r in range(2):
                            kc16 = 2 * m + r
                            ps = ps_mm.tile([P, 512], f32, tag="mm")
                            for j in range(2):
                                nc.tensor.matmul(
                                    ps[:],
                                    x8f_sb[:, 2 * j:2 * j + 2,
                                           kc16 * P:(kc16 + 1) * P],
                                    q8s[h][:, 2 * j:2 * j + 2, qsl],
                                    start=(j == 0), stop=(j == 1),
                                    perf_mode=DR)
                            e_bf = attnp.tile([P, 512], bf16, tag="ebf",
                                              bufs=4)
                            nc.scalar.activation(e_bf[:], ps[:], Exp,
                                                 bias=0.0, scale=ESC)
                            # Centered fp8 weights: split between the scalar
                            # engine (ACT Copy w/ bias, ~710ns) and the
                            # vector engine (~420ns) so neither paces the PE.
                            if kc16 < 4:
                                nc.scalar.activation(e2_t[:, r, :], e_bf[:],
                                                     Copy, bias=-1.0,
                                                     scale=1.0)
                            else:
                                nc.vector.tensor_scalar_sub(
                                    e2_t[:, r, :], e_bf[:], 1.0)
                            if kc16 == 0:
                                nc.vector.tensor_copy(sum_acc[:], e_bf[:])
                            else:
                                nc.vector.tensor_add(sum_acc[:], sum_acc[:],
                                                     e_bf[:])
                        if prev is not None and m == 2:
                            flush(prev, 0)
                        if prev is not None and m == 5:
                            flush(prev, 1)
                    # A@x: yT[kc] = sum_pairs x8k^T e2, kc-major so only
                    # two yT PSUM banks are ever in flight.
                    yt_sb = attnp.tile([P, KC, 512], bf16, tag="ytsb",
                                       bufs=2)
                    for kc in range(KC):
                        ytp = ps_yt.tile([P, 512], f32, tag="yt")
                        for m in range(NPAIR):
                            nc.tensor.matmul(
                                ytp[:],
                                x8k_sb[:, 2 * m:2 * m + 2,
                                       kc * P:(kc + 1) * P],
                                e2s[m][:],
                                start=(m == 0), stop=(m == NPAIR - 1),
                                perf_mode=DR)
                        nc.vector.tensor_scalar_add(
                            yt_sb[:, kc, :], ytp[:], csum_sb[:, kc:kc + 1])
                    prev = [h, qb, yt_sb, sum_acc, []]
            flush(prev, 0)
            flush(prev, 1)

    nc.compile()
    return nc


def _get_nc():
    if "nc" not in _NC_CACHE:
        _NC_CACHE["nc"] = _build_nc()
    return _NC_CACHE["nc"]


def _make_in_maps(x, Wq, Wk, Wv, Wu, bu):
    f32 = np.float32
    bf16 = ml_dtypes.bfloat16
    fp8 = ml_dtypes.float8_e4m3
    inv2 = 1.0 / np.sqrt(K)
    Wq32 = np.asarray(Wq, f32)
    Wk32 = np.asarray(Wk, f32)
    Wv32 = np.asarray(Wv, f32)
    Wu32 = np.asarray(Wu, f32)
    M = np.empty((H, K, K), f32)
    G = np.empty((H, K, K), f32)
    for h in range(H):
        hs = slice(h * K, (h + 1) * K)
        M[h] = (Wq32[:, hs] @ Wk32[:, hs].T) * inv2
        G[h] = (Wv32[:, hs] @ Wu32[hs, :]) * (1.0 / SX)
    bias_bc = np.ascontiguousarray(
        np.broadcast_to((np.asarray(bu, f32) * 0.5)[None, :], (P, K)))
    in_maps = []
    for c in range(NCORES):
        b, r = c // 2, c % 2
        xb = np.asarray(x[b], f32)            # [T, K]
        x8v = np.clip(xb * SX, -240.0, 240.0).astype(fp8)   # [T, K]
        heads = range(r * HL, r * HL + HL)
        Mc = np.concatenate([M[h] for h in heads], axis=1)  # [K, HL*K]
        Gc = np.concatenate([G[h] for h in heads], axis=1)
        # contiguous per-partition layouts
        xT = xb.T.reshape(KC, P, QB, 512).transpose(1, 2, 0, 3)
        x8f = x8v.T.reshape(KC, P, T).transpose(1, 0, 2)
        x8k = x8v.reshape(TC, P, K).transpose(1, 0, 2)
        wm = Mc.reshape(KC, P, HL * K).transpose(1, 0, 2)
        wg = Gc.reshape(KC, P, HL * K).transpose(1, 0, 2)
        # Exact-x column sums (32x scale): the uniform attention component
        # (~90% of the output energy) then sees no fp8 noise at all.
        csum = (xb.sum(axis=0) * SX).reshape(KC, P).T       # [P, KC]
        in_maps.append({
            "xTc": np.ascontiguousarray(xT).astype(bf16),
            "x8f": np.ascontiguousarray(x8f),
            "x8k": np.ascontiguousarray(x8k),
            "wmc": np.ascontiguousarray(wm).astype(bf16),
            "wgc": np.ascontiguousarray(wg).astype(bf16),
            "csum": np.ascontiguousarray(csum),
            "bias_bc": bias_bc,
        })
    return in_maps


def _bf16_to_f32(a):
    # fast bf16 -> f32 (numpy view trick; ml_dtypes elementwise is slow)
    u = np.asarray(a).view(np.uint16).astype(np.uint32) << 16
    return u.view(np.float32)


def _assemble(results):
    out = np.empty((B, T, K), np.float32)
    for b in range(B):
        pe = _bf16_to_f32(results[2 * b]["out"])
        po = _bf16_to_f32(results[2 * b + 1]["out"])
        out[b] = pe + po
    return out


def run_on_hw(x, Wq, Wk, Wv, Wu, bu, trace=False, tmpdir=None):
    from concourse.bass_utils import run_bass_kernel_spmd
    nc = _get_nc()
    in_maps = _make_in_maps(x, Wq, Wk, Wv, Wu, bu)
    res = run_bass_kernel_spmd(nc, in_maps, core_ids=list(range(NCORES)),
                               trace=trace, tmpdir=tmpdir)
    return _assemble(res.results), res


def kernel(x, Wq, Wk, Wv, Wu, bu):
    out, _ = run_on_hw(x, Wq, Wk, Wv, Wu, bu, trace=False)
    return out
